# revision 4
# baseline (speedup 1.0000x reference)
"""KPConv decoder (3 pyramid levels) on 8 Trainium2 NeuronCores.

Strategy (zero-collective, locality sharding):
  *  gather(X, idx) @ W_top == gather(X @ W_top, idx): compute the coarse-side
     matmul FIRST into a small row-major table, then gather rows of the result.
     This cuts FLOPs ~2x and gather bytes 2-4x.
  *  Points at every level are bucketed (on the host) by WHICH CORE owns the
     coarse row their first-neighbor index points at.  Each core then only
     gathers rows from the table shard it computed itself -> no collectives,
     8 fully independent SPMD cores.
  *  Per level (coarse->fine):
        table = prev_latent @ W_top + b           (row-major, DRAM scratch)
        x     = gather(table, idx) + feats @ W_bot
        y     = LeakyReLU(GroupNorm(x) * gamma + beta)     (last level: y = x)
  *  Host un-permutes the per-core row blocks back to the original order.

All device matmuls are fp32.  Activations are fed feature-major (host
pre-transpose) so the contraction dim lands on partitions; intermediate
latents are PE-transposed on-chip to feed the next table matmul.
"""

import numpy as np

NCORES = 8
P = 128
GROUPS = 32
EPS = 1e-5
SLOPE = 0.1


# ---------------------------------------------------------------- host helpers

def _ceil_mult(x, m):
    return max(m, (int(x) + m - 1) // m * m)


def _bucket(core_of, n_cores=NCORES):
    """perm (stable-groups items by core), counts, offsets, and each item's
    position inside its core block."""
    n = core_of.shape[0]
    perm = np.argsort(core_of, kind="stable")
    counts = np.bincount(core_of, minlength=n_cores).astype(np.int64)
    offs = np.zeros(n_cores + 1, np.int64)
    offs[1:] = np.cumsum(counts)
    pos = np.empty(n, np.int64)
    pos[perm] = np.arange(n, dtype=np.int64) - np.repeat(offs[:-1], counts)
    return perm, counts, offs, pos


def _prep_feats(feats, sel, S):
    """Rows `sel` of feats [N, C] -> feature-major [128, C//128, S] (padded)."""
    C = feats.shape[1]
    k = C // P
    a = np.zeros((S, C), np.float32)
    a[: len(sel)] = feats[sel]
    return np.ascontiguousarray(a.T.reshape(k, P, S).transpose(1, 0, 2))


def _prep_idx(local_idx, S):
    """int16 gather indices wrapped [16, S/16] and replicated to 128 parts."""
    ii = np.zeros(S, np.int16)
    ii[: len(local_idx)] = local_idx.astype(np.int16)
    w = ii.reshape(S // 16, 16).T  # [16, S/16]
    return np.ascontiguousarray(np.tile(w, (8, 1)))  # [128, S/16]


def _prep_w(W):
    """[Cin, Cout] -> [128, Cin//128, Cout] (contraction chunk on partitions)."""
    Cin, Cout = W.shape
    k = Cin // P
    return np.ascontiguousarray(W.reshape(k, P, Cout).transpose(1, 0, 2))


# ---------------------------------------------------------------- device build

def _build_program(dims):
    import concourse.bacc as bacc
    import concourse.mybir as mybir
    import concourse.tile as tile
    from concourse.masks import make_identity

    f32 = mybir.dt.float32
    i16 = mybir.dt.int16
    S1, S2, S3, T4P = dims["S1"], dims["S2"], dims["S3"], dims["T4P"]

    nc = bacc.Bacc("TRN2", target_bir_lowering=False, debug=False,
                   enable_asserts=False, num_devices=NCORES)

    # --- I/O ---------------------------------------------------------------
    a4_d = nc.dram_tensor("a4", [P, 4, T4P], f32, kind="ExternalInput")
    a3_d = nc.dram_tensor("a3", [P, 2, S3], f32, kind="ExternalInput")
    a2_d = nc.dram_tensor("a2", [P, 2, S2], f32, kind="ExternalInput")
    a1_d = nc.dram_tensor("a1", [P, 1, S1], f32, kind="ExternalInput")
    i3_d = nc.dram_tensor("i3", [P, S3 // 16], i16, kind="ExternalInput")
    i2_d = nc.dram_tensor("i2", [P, S2 // 16], i16, kind="ExternalInput")
    i1_d = nc.dram_tensor("i1", [P, S1 // 16], i16, kind="ExternalInput")
    w3t_d = nc.dram_tensor("w3t", [P, 4, 512], f32, kind="ExternalInput")
    w3b_d = nc.dram_tensor("w3b", [P, 2, 512], f32, kind="ExternalInput")
    w2t_d = nc.dram_tensor("w2t", [P, 4, 256], f32, kind="ExternalInput")
    w2b_d = nc.dram_tensor("w2b", [P, 2, 256], f32, kind="ExternalInput")
    w1t_d = nc.dram_tensor("w1t", [P, 2, 128], f32, kind="ExternalInput")
    w1b_d = nc.dram_tensor("w1b", [P, 1, 128], f32, kind="ExternalInput")
    b3_d = nc.dram_tensor("b3v", [512], f32, kind="ExternalInput")
    g3_d = nc.dram_tensor("g3v", [512], f32, kind="ExternalInput")
    e3_d = nc.dram_tensor("e3v", [512], f32, kind="ExternalInput")
    b2_d = nc.dram_tensor("b2v", [256], f32, kind="ExternalInput")
    g2_d = nc.dram_tensor("g2v", [256], f32, kind="ExternalInput")
    e2_d = nc.dram_tensor("e2v", [256], f32, kind="ExternalInput")
    b1_d = nc.dram_tensor("b1v", [128], f32, kind="ExternalInput")
    o3_d = nc.dram_tensor("o3", [S3, 512], f32, kind="ExternalOutput")
    o2_d = nc.dram_tensor("o2", [S2, 256], f32, kind="ExternalOutput")
    o1_d = nc.dram_tensor("o1", [S1, 128], f32, kind="ExternalOutput")

    import concourse.bass as bass

    def bcast_ap(dram_t, C):
        ap = dram_t.ap()
        return bass.AP(tensor=ap.tensor, offset=ap.offset, ap=[[0, P]] + list(ap.ap))

    with tile.TileContext(nc) as tc:
        from contextlib import ExitStack

        with ExitStack() as top:
            consts = top.enter_context(tc.tile_pool(name="consts", bufs=1))
            drp = top.enter_context(tc.tile_pool(name="drscr", bufs=1, space="DRAM"))

            # resident weights / vectors / indices
            w3t_s = consts.tile([P, 4, 512], f32)
            nc.sync.dma_start(w3t_s[:], w3t_d.ap())
            w3b_s = consts.tile([P, 2, 512], f32)
            nc.sync.dma_start(w3b_s[:], w3b_d.ap())
            w2t_s = consts.tile([P, 4, 256], f32)
            nc.sync.dma_start(w2t_s[:], w2t_d.ap())
            w2b_s = consts.tile([P, 2, 256], f32)
            nc.sync.dma_start(w2b_s[:], w2b_d.ap())
            w1t_s = consts.tile([P, 2, 128], f32)
            nc.sync.dma_start(w1t_s[:], w1t_d.ap())
            w1b_s = consts.tile([P, 1, 128], f32)
            nc.sync.dma_start(w1b_s[:], w1b_d.ap())
            a4_s = consts.tile([P, 4, T4P], f32)
            nc.sync.dma_start(a4_s[:], a4_d.ap())

            b3r = consts.tile([P, 512], f32)
            nc.gpsimd.dma_start(b3r[:], bcast_ap(b3_d, 512))
            g3r = consts.tile([P, 512], f32)
            nc.gpsimd.dma_start(g3r[:], bcast_ap(g3_d, 512))
            e3r = consts.tile([P, 512], f32)
            nc.gpsimd.dma_start(e3r[:], bcast_ap(e3_d, 512))
            b2r = consts.tile([P, 256], f32)
            nc.gpsimd.dma_start(b2r[:], bcast_ap(b2_d, 256))
            g2r = consts.tile([P, 256], f32)
            nc.gpsimd.dma_start(g2r[:], bcast_ap(g2_d, 256))
            e2r = consts.tile([P, 256], f32)
            nc.gpsimd.dma_start(e2r[:], bcast_ap(e2_d, 256))
            b1r = consts.tile([P, 128], f32)
            nc.gpsimd.dma_start(b1r[:], bcast_ap(b1_d, 128))

            i3_s = consts.tile([P, S3 // 16], i16)
            nc.sync.dma_start(i3_s[:], i3_d.ap())
            i2_s = consts.tile([P, S2 // 16], i16)
            nc.sync.dma_start(i2_s[:], i2_d.ap())
            i1_s = consts.tile([P, S1 // 16], i16)
            nc.sync.dma_start(i1_s[:], i1_d.ap())

            ident = consts.tile([P, P], f32)
            make_identity(nc, ident[:])
            eps_t = consts.tile([P, 1], f32)
            nc.vector.memset(eps_t[:], EPS)

            y4_t = drp.tile([T4P, 512], f32)
            z3_t = drp.tile([S3, 256], f32)
            z2_t = drp.tile([S2, 128], f32)

            # ---------------- stage A: Y4 = feats_s4 @ W3_top + b3 ----------
            with ExitStack() as st:
                psA = st.enter_context(tc.tile_pool(name="psA", bufs=2, space="PSUM"))
                evA = st.enter_context(tc.tile_pool(name="evA", bufs=2))
                for mt in range(T4P // P):
                    ps = psA.tile([P, 512], f32)
                    for kt in range(4):
                        nc.tensor.matmul(ps[:], a4_s[:, kt, mt * P:(mt + 1) * P],
                                         w3t_s[:, kt, :],
                                         start=(kt == 0), stop=(kt == 3))
                    ev = evA.tile([P, 512], f32)
                    nc.vector.tensor_add(ev[:], ps[:], b3r[:])
                    nc.sync.dma_start(y4_t[mt * P:(mt + 1) * P, :], ev[:])

            # ---------------- stages B/C: unary-block levels -----------------
            def unary_level(S, CH, k_in, Cmid, a_dram, w_bot, idx_s, table, telem,
                            out_dram, gam, bet, znext, k_mid, Cnext, wnext, bnext):
                d = Cmid // GROUPS
                with ExitStack() as st:
                    ach = st.enter_context(tc.tile_pool(name="ach", bufs=3))
                    gpool = st.enter_context(tc.tile_pool(name="gpool", bufs=2))
                    psum = st.enter_context(tc.tile_pool(name="psum", bufs=2, space="PSUM"))
                    psz = st.enter_context(tc.tile_pool(name="psz", bufs=2, space="PSUM"))
                    pst = st.enter_context(tc.tile_pool(name="pst", bufs=2, space="PSUM"))
                    xp = st.enter_context(tc.tile_pool(name="xp", bufs=3))
                    yp = st.enter_context(tc.tile_pool(name="yp", bufs=3))
                    sm = st.enter_context(tc.tile_pool(name="sm", bufs=4))
                    ltp = st.enter_context(tc.tile_pool(name="ltp", bufs=2))
                    nch = (S + CH - 1) // CH
                    for c in range(nch):
                        n_c = min(CH, S - c * CH)
                        nt = n_c // P
                        a_ch = ach.tile([P, k_in, CH], f32)
                        nc.sync.dma_start(a_ch[:, :, :n_c],
                                          a_dram.ap()[:, :, c * CH:c * CH + n_c])
                        gt = gpool.tile([P, CH // P, Cmid], f32)
                        GSUB = 512  # >1024-idx gathers are fatal on HW
                        for s in range(0, n_c, GSUB):
                            n_s = min(GSUB, n_c - s)
                            nc.gpsimd.dma_gather(
                                gt[:, s // P:(s + n_s) // P, :], table[:],
                                idx_s[:, (c * CH + s) // 16:(c * CH + s + n_s) // 16],
                                n_s, n_s, Cmid)
                        for t in range(nt):
                            r0 = c * CH + t * P
                            ps = psum.tile([P, Cmid], f32)
                            for kt in range(k_in):
                                nc.tensor.matmul(ps[:], a_ch[:, kt, t * P:(t + 1) * P],
                                                 w_bot[:, kt, :],
                                                 start=(kt == 0), stop=(kt == k_in - 1))
                            x = xp.tile([P, GROUPS, d], f32)
                            xf = x[:].rearrange("p g d -> p (g d)")
                            nc.vector.tensor_add(xf, ps[:], gt[:, t, :])
                            # --- GroupNorm stats (E[x^2]-E[x]^2) ---
                            s1 = sm.tile([P, GROUPS], f32)
                            nc.vector.reduce_sum(s1[:], x[:], axis=AXX)
                            xsq = yp.tile([P, GROUPS, d], f32)
                            nc.scalar.activation(
                                xsq[:].rearrange("p g d -> p (g d)"), xf,
                                func=SQUARE)
                            s2 = sm.tile([P, GROUPS], f32)
                            nc.vector.reduce_sum(s2[:], xsq[:], axis=AXX)
                            mean = sm.tile([P, GROUPS], f32)
                            nc.scalar.mul(mean[:], s1[:], 1.0 / d)
                            msq = sm.tile([P, GROUPS], f32)
                            nc.vector.tensor_mul(msq[:], mean[:], mean[:])
                            var = sm.tile([P, GROUPS], f32)
                            nc.vector.tensor_scalar_mul(var[:], s2[:], 1.0 / d)
                            nc.vector.tensor_tensor(var[:], var[:], msq[:], op=SUB)
                            std = sm.tile([P, GROUPS], f32)
                            nc.scalar.activation(std[:], var[:], func=SQRT,
                                                 bias=eps_t[:], scale=1.0)
                            r = sm.tile([P, GROUPS], f32)
                            nc.vector.reciprocal(r[:], std[:])
                            mr = sm.tile([P, GROUPS], f32)
                            nc.vector.tensor_mul(mr[:], mean[:], r[:])
                            # --- normalize + affine + leaky ---
                            y = yp.tile([P, GROUPS, d], f32)
                            nc.vector.tensor_tensor(
                                y[:], x[:],
                                r[:].unsqueeze(2).to_broadcast([P, GROUPS, d]),
                                op=MULT)
                            nc.vector.tensor_tensor(
                                y[:], y[:],
                                mr[:].unsqueeze(2).to_broadcast([P, GROUPS, d]),
                                op=SUB)
                            yf = y[:].rearrange("p g d -> p (g d)")
                            if gam is not None:
                                nc.vector.tensor_mul(yf, yf, gam[:])
                            if bet is not None:
                                nc.vector.tensor_add(yf, yf, bet[:])
                            t01 = yp.tile([P, Cmid], f32)
                            nc.scalar.mul(t01[:], yf, SLOPE)
                            yo = yp.tile([P, Cmid], f32)
                            nc.vector.tensor_tensor(yo[:], yf, t01[:], op=MAX)
                            nc.sync.dma_start(out_dram.ap()[r0:r0 + P, :], yo[:])
                            # --- transpose + next-level table ---
                            ltT = ltp.tile([P, k_mid, P], f32)
                            for q in range(k_mid):
                                pt = pst.tile([P, P], f32)
                                nc.tensor.transpose(pt[:], yo[:, q * P:(q + 1) * P],
                                                    ident[:])
                                nc.any.tensor_copy(ltT[:, q, :], pt[:])
                            pz = psz.tile([P, Cnext], f32)
                            for q in range(k_mid):
                                nc.tensor.matmul(pz[:], ltT[:, q, :], wnext[:, q, :],
                                                 start=(q == 0), stop=(q == k_mid - 1))
                            ez = yp.tile([P, Cnext], f32)
                            nc.vector.tensor_add(ez[:], pz[:], bnext[:])
                            nc.sync.dma_start(znext[r0:r0 + P, :], ez[:])

            import concourse.mybir as mybir
            AXX = mybir.AxisListType.X
            SUB = mybir.AluOpType.subtract
            MULT = mybir.AluOpType.mult
            MAX = mybir.AluOpType.max
            SQUARE = mybir.ActivationFunctionType.Square
            SQRT = mybir.ActivationFunctionType.Sqrt

            gam3 = None if dims["g3_triv"] else g3r
            bet3 = None if dims["e3_triv"] else e3r
            gam2 = None if dims["g2_triv"] else g2r
            bet2 = None if dims["e2_triv"] else e2r

            unary_level(S3, 1024, 2, 512, a3_d, w3b_s, i3_s, y4_t[:], 512,
                        o3_d, gam3, bet3, z3_t[:], 4, 256, w2t_s, b2r)
            unary_level(S2, 2048, 2, 256, a2_d, w2b_s, i2_s, z3_t[:], 256,
                        o2_d, gam2, bet2, z2_t[:], 2, 128, w1t_s, b1r)

            # ---------------- stage D: level 1 (linear only) -----------------
            with ExitStack() as st:
                ach = st.enter_context(tc.tile_pool(name="ach1", bufs=3))
                gpool = st.enter_context(tc.tile_pool(name="gp1", bufs=2))
                psum = st.enter_context(tc.tile_pool(name="ps1", bufs=2, space="PSUM"))
                yp = st.enter_context(tc.tile_pool(name="yp1", bufs=3))
                CH = 2048
                nch = (S1 + CH - 1) // CH
                for c in range(nch):
                    n_c = min(CH, S1 - c * CH)
                    nt = n_c // P
                    a_ch = ach.tile([P, 1, CH], f32)
                    nc.sync.dma_start(a_ch[:, :, :n_c],
                                      a1_d.ap()[:, :, c * CH:c * CH + n_c])
                    gt = gpool.tile([P, CH // P, 128], f32)
                    GSUB = 512
                    for s in range(0, n_c, GSUB):
                        n_s = min(GSUB, n_c - s)
                        nc.gpsimd.dma_gather(
                            gt[:, s // P:(s + n_s) // P, :], z2_t[:],
                            i1_s[:, (c * CH + s) // 16:(c * CH + s + n_s) // 16],
                            n_s, n_s, 128)
                    for t in range(nt):
                        r0 = c * CH + t * P
                        ps = psum.tile([P, 128], f32)
                        nc.tensor.matmul(ps[:], a_ch[:, 0, t * P:(t + 1) * P],
                                         w1b_s[:, 0, :], start=True, stop=True)
                        yo = yp.tile([P, 128], f32)
                        nc.vector.tensor_add(yo[:], ps[:], gt[:, t, :])
                        nc.sync.dma_start(o1_d.ap()[r0:r0 + P, :], yo[:])

    nc.compile()
    return nc


# -------------------------------------------------------------------- kernel

def kernel(**inputs):
    feats_s1 = np.asarray(inputs["feats_s1"], np.float32)
    feats_s2 = np.asarray(inputs["feats_s2"], np.float32)
    feats_s3 = np.asarray(inputs["feats_s3"], np.float32)
    feats_s4 = np.asarray(inputs["feats_s4"], np.float32)
    up0 = np.asarray(inputs["up0"])
    up1 = np.asarray(inputs["up1"])
    up2 = np.asarray(inputs["up2"])
    W3 = np.asarray(inputs["W3"], np.float32)
    b3 = np.asarray(inputs["b3"], np.float32)
    g3 = np.asarray(inputs["g3"], np.float32)
    beta3 = np.asarray(inputs["beta3"], np.float32)
    W2 = np.asarray(inputs["W2"], np.float32)
    b2 = np.asarray(inputs["b2"], np.float32)
    g2 = np.asarray(inputs["g2"], np.float32)
    beta2 = np.asarray(inputs["beta2"], np.float32)
    W1 = np.asarray(inputs["W1"], np.float32)
    b1 = np.asarray(inputs["b1"], np.float32)

    N1 = feats_s1.shape[0]
    N2 = feats_s2.shape[0]
    N3 = feats_s3.shape[0]
    N4 = feats_s4.shape[0]
    idx3 = np.asarray(up2[:, 0], np.int64)
    idx2 = np.asarray(up1[:, 0], np.int64)
    idx1 = np.asarray(up0[:, 0], np.int64)

    # ---- locality bucketing ------------------------------------------------
    T4 = (N4 + NCORES - 1) // NCORES                      # feats_s4 rows/core
    core3 = idx3 // T4
    perm3, counts3, offs3, pos3 = _bucket(core3)
    S3 = _ceil_mult(counts3.max(), 128)

    core2 = core3[idx2]
    perm2, counts2, offs2, pos2 = _bucket(core2)
    S2 = _ceil_mult(counts2.max(), 128)

    core1 = core2[idx1]
    perm1, counts1, offs1, pos1 = _bucket(core1)
    S1 = _ceil_mult(counts1.max(), 128)

    T4P = _ceil_mult(T4, 128)

    dims = dict(S1=S1, S2=S2, S3=S3, T4P=T4P,
                g3_triv=bool(np.all(g3 == 1.0)), e3_triv=bool(np.all(beta3 == 0.0)),
                g2_triv=bool(np.all(g2 == 1.0)), e2_triv=bool(np.all(beta2 == 0.0)))

    nc = _build_program(dims)

    # ---- per-core input maps ----------------------------------------------
    w3t = _prep_w(W3[:512])
    w3b = _prep_w(W3[512:])
    w2t = _prep_w(W2[:512])
    w2b = _prep_w(W2[512:])
    w1t = _prep_w(W1[:256])
    w1b = _prep_w(W1[256:])

    in_maps = []
    for c in range(NCORES):
        s3c = perm3[offs3[c]:offs3[c + 1]]
        s2c = perm2[offs2[c]:offs2[c + 1]]
        s1c = perm1[offs1[c]:offs1[c + 1]]
        f4 = np.zeros((T4P, feats_s4.shape[1]), np.float32)
        rows = feats_s4[c * T4: min((c + 1) * T4, N4)]
        f4[: len(rows)] = rows
        a4 = np.ascontiguousarray(
            f4.T.reshape(4, P, T4P).transpose(1, 0, 2))
        in_maps.append({
            "a4": a4,
            "a3": _prep_feats(feats_s3, s3c, S3),
            "a2": _prep_feats(feats_s2, s2c, S2),
            "a1": _prep_feats(feats_s1, s1c, S1),
            "i3": _prep_idx(idx3[s3c] - c * T4, S3),
            "i2": _prep_idx(pos3[idx2[s2c]], S2),
            "i1": _prep_idx(pos2[idx1[s1c]], S1),
            "w3t": w3t, "w3b": w3b, "w2t": w2t, "w2b": w2b,
            "w1t": w1t, "w1b": w1b,
            "b3v": b3, "g3v": g3, "e3v": beta3,
            "b2v": b2, "g2v": g2, "e2v": beta2, "b1v": b1,
        })

    from concourse.bass_utils import run_bass_kernel_spmd
    res = run_bass_kernel_spmd(nc, in_maps, core_ids=list(range(NCORES)))
    kernel.last_results = res
    kernel.last_nc = nc

    # ---- un-permute outputs ------------------------------------------------
    lat3 = np.empty((N3, 512), np.float32)
    lat2 = np.empty((N2, 256), np.float32)
    lat1 = np.empty((N1, 128), np.float32)
    for c in range(NCORES):
        lat3[perm3[offs3[c]:offs3[c + 1]]] = res.results[c]["o3"][: counts3[c]]
        lat2[perm2[offs2[c]:offs2[c + 1]]] = res.results[c]["o2"][: counts2[c]]
        lat1[perm1[offs1[c]:offs1[c + 1]]] = res.results[c]["o1"][: counts1[c]]
    return (lat1, lat2, lat3)
